# revision 9
# baseline (speedup 1.0000x reference)
"""Trainium2 Bass kernel for ClebschCombiningSingleUnrolled (segment_reduce).

out[mu_k] += mult_k * X1[m1_k] * X2[m2_k]   summed over k, per (n, d) element.

Shapes (hardcoded): X1, X2: [9, 4096, 256] f32; index lists: [100]; out: [9, 4096, 256] f32.
Sharding: N (dim 1) split across 8 NeuronCores; index math is host-side.
"""

import os
import sys
import functools

import numpy as np

sys.path.insert(0, "/opt/trn_rl_repo")
os.environ.setdefault("MYCRO_LOCAL_CACHE", "1")

import concourse.bass as bass  # noqa: E402
import concourse.bacc as bacc  # noqa: E402
import concourse.tile as tile  # noqa: E402
from concourse import mybir  # noqa: E402
from concourse.bass_utils import run_bass_kernel_spmd  # noqa: E402

M = 9
N = 4096
D = 256
K = 100
NCORES = 8
NS = N // NCORES  # 512 environment pairs per core
F32 = mybir.dt.float32

MULT = mybir.AluOpType.mult
ADD = mybir.AluOpType.add


def _plan(m1, m2, mu, mult):
    """Group the K terms: merge exact (a,b,mu) duplicates, then group by (a,b) pair.

    Returns list of (a, b, [(mu, w), ...]).
    """
    merged = {}
    for a, b, m, w in zip(m1, m2, mu, mult):
        key = (int(a), int(b), int(m))
        merged[key] = merged.get(key, 0.0) + float(w)
    pairs = {}
    for (a, b, m), w in merged.items():
        pairs.setdefault((a, b), []).append((m, w))
    return [(a, b, uses) for (a, b), uses in sorted(pairs.items())]


def _build_dve_kernel(m1, m2, mu, mult):
    """Phase-1 kernel: layout C (n on partitions, (m, d) on free), all work on DVE.

    Exact fp32. Per 128-row n-chunk: memset acc, one product per unique (a,b)
    pair, one fused scale-and-accumulate (scalar_tensor_tensor) per term.
    """
    plan = _plan(m1, m2, mu, mult)

    nc = bacc.Bacc(trn_type="TRN2")
    x1_d = nc.dram_tensor("X1", [M, NS, D], F32, kind="ExternalInput")
    x2_d = nc.dram_tensor("X2", [M, NS, D], F32, kind="ExternalInput")
    out_d = nc.dram_tensor("OUT", [M, NS, D], F32, kind="ExternalOutput")

    n_chunks = NS // 128

    with tile.TileContext(nc) as tc:
        with (
            tc.tile_pool(name="io", bufs=2) as io_pool,
            tc.tile_pool(name="acc", bufs=2) as acc_pool,
            tc.tile_pool(name="tmp", bufs=2) as tmp_pool,
        ):
            for c in range(n_chunks):
                n0 = c * 128
                x1t = io_pool.tile([128, M, D], F32, tag="x1t")
                nc.gpsimd.dma_start(
                    x1t[:], x1_d[:, n0 : n0 + 128, :].rearrange("m n d -> n m d")
                )
                x2t = io_pool.tile([128, M, D], F32, tag="x2t")
                nc.gpsimd.dma_start(
                    x2t[:], x2_d[:, n0 : n0 + 128, :].rearrange("m n d -> n m d")
                )

                # Absorb each DMA-completion wait into its own 1-element DVE op:
                # the TensorTensor ISA struct can't encode 2 sem waits, so the
                # first real consumer of (x1t, x2t) must not be the one waiting.
                sink = tmp_pool.tile([1, 2], F32, tag="sink", name="sink")
                nc.vector.tensor_copy(sink[:, 0:1], x1t[0:1, 0, 0:1])
                nc.vector.tensor_copy(sink[:, 1:2], x2t[0:1, 0, 0:1])

                acc = [
                    acc_pool.tile([128, D], F32, tag=f"acc{m}", name=f"acc{m}")
                    for m in range(M)
                ]
                written = [False] * M

                for a, b, uses in plan:
                    if len(uses) == 1:
                        m, w = uses[0]
                        if not written[m]:
                            nc.vector.scalar_tensor_tensor(
                                acc[m][:], x1t[:, a, :], float(w), x2t[:, b, :],
                                MULT, MULT,
                            )
                            written[m] = True
                        else:
                            tmp = tmp_pool.tile([128, D], F32)
                            nc.vector.scalar_tensor_tensor(
                                tmp[:], x1t[:, a, :], float(w), x2t[:, b, :],
                                MULT, MULT,
                            )
                            nc.vector.scalar_tensor_tensor(
                                acc[m][:], tmp[:], 1.0, acc[m][:], MULT, ADD
                            )
                    else:
                        tmp = tmp_pool.tile([128, D], F32)
                        nc.vector.tensor_mul(tmp[:], x1t[:, a, :], x2t[:, b, :])
                        for m, w in uses:
                            if not written[m]:
                                nc.vector.tensor_scalar_mul(
                                    acc[m][:], tmp[:], float(w)
                                )
                                written[m] = True
                            else:
                                nc.vector.scalar_tensor_tensor(
                                    acc[m][:], tmp[:], float(w), acc[m][:], MULT, ADD
                                )

                for m in range(M):
                    if not written[m]:
                        nc.vector.memset(acc[m][:], 0.0)
                    nc.gpsimd.dma_start(out_d[m, n0 : n0 + 128, :], acc[m][:])

    nc.compile()
    return nc


_CACHE = {}


def _get_nc(key, builder, *args):
    if key not in _CACHE:
        _CACHE[key] = builder(*args)
    return _CACHE[key]


def kernel(X1, X2, m1_aligned, m2_aligned, mu, multipliers):
    X1 = np.ascontiguousarray(X1, dtype=np.float32)
    X2 = np.ascontiguousarray(X2, dtype=np.float32)
    m1 = [int(v) for v in np.asarray(m1_aligned)]
    m2 = [int(v) for v in np.asarray(m2_aligned)]
    mus = [int(v) for v in np.asarray(mu)]
    mult = [float(v) for v in np.asarray(multipliers, dtype=np.float32)]

    key = ("dve1", tuple(m1), tuple(m2), tuple(mus), tuple(mult))
    nc = _get_nc(key, _build_dve_kernel, m1, m2, mus, mult)

    in_maps = []
    for c in range(NCORES):
        sl = slice(c * NS, (c + 1) * NS)
        in_maps.append(
            {
                "X1": np.ascontiguousarray(X1[:, sl, :]),
                "X2": np.ascontiguousarray(X2[:, sl, :]),
            }
        )

    res = run_bass_kernel_spmd(nc, in_maps, core_ids=list(range(NCORES)))
    out = np.concatenate([res.results[c]["OUT"] for c in range(NCORES)], axis=1)
    return out
